# revision 19
# baseline (speedup 1.0000x reference)
"""Multi-head attention TRN2 Bass kernel (8-core SPMD).

Problem: nn_MultiHeadAttention — B=4, S=2048, H=512, NH=8, HD=64.
Returns (output [B,S,H], attention [B,NH,S,S]), matching the reference
nn.MultiheadAttention in eval mode.

Sharding: (batch, head-group) over 8 cores — core c handles batch c//2 and
heads (c%2)*4 .. (c%2)*4+4.  Each core computes its QKV projections for its
head slice, scores + softmax (exact fp32, written to DRAM), attention@V, and
a partial output projection.  Host sums the two partial output projections
per batch and adds bo.

Kernel internals (per core):
  - All matmuls run as float32r (full-rate fp32 path on the PE).
  - x and the weight slices are transposed on-chip via TensorE.
  - scores are computed twice: natural [q,k] orientation for the softmax /
    attention output (rowsum via ACT accum_out), and transposed [k,q]
    orientation (exp'd again) to feed attention@V with k on partitions.
  - context is produced transposed [o,q]; normalization 1/Z is applied via a
    partition-broadcast multiply, with 1/Z row vectors built by tiny TensorE
    transposes of the per-partition 1/Z columns.
"""

import os
import sys

import numpy as np

for _p in ("/opt/trn_rl_repo", "/root/.axon_site/_ro/trn_rl_repo"):
    if os.path.isdir(_p) and _p not in sys.path:
        sys.path.append(_p)

import concourse.bacc as bacc
import concourse.bass as bass
import concourse.tile as tile
from concourse import bass_utils, mybir

B, S, H, NH, HD = 4, 2048, 512, 8, 64
NCORES = 8
NHL = NH // 2  # heads per core
OH = NHL * HD  # per-core projection width (256)

F32 = mybir.dt.float32
F32R = mybir.dt.float32r
EXP = mybir.ActivationFunctionType.Exp


def declare_io(nc, s, h, oh):
    nhl = oh // HD
    return dict(
        xq=nc.dram_tensor("xq", [s, h], F32, kind="ExternalInput").ap(),
        xk=nc.dram_tensor("xk", [s, h], F32, kind="ExternalInput").ap(),
        xv=nc.dram_tensor("xv", [s, h], F32, kind="ExternalInput").ap(),
        wq=nc.dram_tensor("wq", [oh, h], F32, kind="ExternalInput").ap(),
        wk=nc.dram_tensor("wk", [oh, h], F32, kind="ExternalInput").ap(),
        wv=nc.dram_tensor("wv", [oh, h], F32, kind="ExternalInput").ap(),
        wo=nc.dram_tensor("wo", [h, oh], F32, kind="ExternalInput").ap(),
        bq=nc.dram_tensor("bq", [oh], F32, kind="ExternalInput").ap(),
        bk=nc.dram_tensor("bk", [oh], F32, kind="ExternalInput").ap(),
        bv=nc.dram_tensor("bv", [oh], F32, kind="ExternalInput").ap(),
        ident=nc.dram_tensor("ident", [128, 128], F32, kind="ExternalInput").ap(),
        att=nc.dram_tensor("att", [nhl, s, s], F32, kind="ExternalOutput").ap(),
        outp=nc.dram_tensor("outp", [s, h], F32, kind="ExternalOutput").ap(),
    )


def build_attn(nc, tc, s, h, oh, io=None, skip=()):
    """Emit the per-core attention program. s=seq len, h=model dim,
    oh=per-core head-slice width (NHL*64)."""
    hd = HD
    nhl = oh // hd
    st_n = s // 128  # s-tiles
    ht_n = h // 128  # h-tiles
    ot_n = oh // 128  # o-tiles
    qc = min(512, s)  # q-chunk for the transposed-scores path
    nqc = s // qc
    qt_per_c = qc // 128
    kt_n = s // 128
    sc = min(512, s)  # moving-dim chunk for projections / scores
    nsc = s // sc

    if io is None:
        io = declare_io(nc, s, h, oh)
    xq, xk, xv = io["xq"], io["xk"], io["xv"]
    wq, wk, wv, wo = io["wq"], io["wk"], io["wv"], io["wo"]
    bq, bk, bv = io["bq"], io["bk"], io["bv"]
    ident, att, outp = io["ident"], io["att"], io["outp"]

    scale = hd**-0.5

    cpool = tc.alloc_tile_pool(name="const", bufs=1)
    ppool = tc.alloc_tile_pool(name="persist", bufs=1)

    id_sb = cpool.tile([128, 128], F32, tag="ident")
    nc.sync.dma_start(id_sb, ident)
    ones_f = cpool.tile([1, 128], F32, tag="onesf")
    nc.vector.memset(ones_f, 1.0)
    ones_sb = cpool.tile([1, 128], F32R, tag="ones")
    nc.vector.tensor_copy(ones_sb, ones_f)
    bv_sb = cpool.tile([1, oh], F32, tag="bv")
    nc.sync.dma_start(bv_sb, bv[None, :])
    bv_r = cpool.tile([1, oh], F32R, tag="bvr")
    nc.vector.tensor_copy(bv_r, bv_sb)
    bq_sb = []
    bk_sb = []
    for ot in range(ot_n):
        tq = cpool.tile([128, 1], F32, tag=f"bq{ot}", name=f"bq{ot}")
        nc.sync.dma_start(tq, bq[ot * 128 : (ot + 1) * 128][:, None])
        bq_sb.append(tq)
        tk = cpool.tile([128, 1], F32, tag=f"bk{ot}", name=f"bk{ot}")
        nc.sync.dma_start(tk, bk[ot * 128 : (ot + 1) * 128][:, None])
        bk_sb.append(tk)

    # Persistent SBUF: projected tensors + context.
    # qTh[h]/kTh[h] are [128, s] with the head's 64 dims in rows 0:64 and
    # zeros in rows 64:128, so score matmuls contract over K=128 (2x faster
    # on the PE than K=64).
    qTh = [ppool.tile([128, s], F32R, tag=f"qTh{i}", name=f"qTh{i}") for i in range(nhl)]
    kTh = [ppool.tile([128, s], F32R, tag=f"kTh{i}", name=f"kTh{i}") for i in range(nhl)]
    zpad = cpool.tile([64, s], F32, tag="zpad")
    nc.vector.memset(zpad, 0.0)
    for i in range(nhl):
        nc.vector.tensor_copy(qTh[i][64:128, :], zpad)
        nc.vector.tensor_copy(kTh[i][64:128, :], zpad)
    v_sb = [ppool.tile([128, oh], F32R, tag=f"v{t}", name=f"v{t}") for t in range(st_n)]
    ctx = [ppool.tile([128, s], F32R, tag=f"ctx{t}", name=f"ctx{t}") for t in range(ot_n)]
    woT = [ppool.tile([128, h], F32R, tag=f"woT{t}", name=f"woT{t}") for t in range(ot_n)]

    # ---- weight transposes: wq/wk/wv [oh,h] -> [h,oh]; wo [h,oh] -> [oh,h]
    with (
        tc.tile_pool(name="wsb", bufs=2) as wpool,
        tc.tile_pool(name="wps", bufs=2, space="PSUM") as wpsum,
    ):
        wqT = [wpool.tile([128, oh], F32R, tag=f"wqT{t}", name=f"wqT{t}") for t in range(ht_n)]
        wkT = [wpool.tile([128, oh], F32R, tag=f"wkT{t}", name=f"wkT{t}") for t in range(ht_n)]
        wvT = [wpool.tile([128, oh], F32R, tag=f"wvT{t}", name=f"wvT{t}") for t in range(ht_n)]
        for w_dram, wT in ((wq, wqT), (wk, wkT), (wv, wvT)):
            for ot in range(ot_n):
                wn = wpool.tile([128, h], F32, tag="wn")
                nc.sync.dma_start(wn, w_dram[ot * 128 : (ot + 1) * 128, :])
                for ht in range(ht_n):
                    ps = wpsum.tile([128, 128], F32, tag="wt")
                    nc.tensor.transpose(ps, wn[:, ht * 128 : (ht + 1) * 128], id_sb)
                    nc.scalar.copy(wT[ht][:, ot * 128 : (ot + 1) * 128], ps)
        for ht in range(ht_n):
            wn = wpool.tile([128, h], F32, tag="wn")
            nc.sync.dma_start(wn[:, :oh], wo[ht * 128 : (ht + 1) * 128, :])
            for ot in range(ot_n):
                ps = wpsum.tile([128, 128], F32, tag="wt")
                nc.tensor.transpose(ps, wn[:, ot * 128 : (ot + 1) * 128], id_sb)
                nc.scalar.copy(woT[ot][:, ht * 128 : (ht + 1) * 128], ps)

        # ---- x transpose + projections (one input tensor at a time)
        for which, x_dram, wT in (("q", xq, wqT), ("k", xk, wkT), ("v", xv, wvT)):
            with (
                tc.tile_pool(name=f"xT_{which}", bufs=1) as xtp,
                tc.tile_pool(name=f"xn_{which}", bufs=8) as xnp,
                tc.tile_pool(name=f"xps_{which}", bufs=3, space="PSUM") as xps,
                tc.tile_pool(name=f"pps_{which}", bufs=2, space="PSUM") as pps,
            ):
                xT = [xtp.tile([128, s], F32R, tag=f"xT{t}", name=f"xT{t}") for t in range(ht_n)]
                g = min(4, st_n)
                for stg in range(st_n // g):
                    xns = []
                    for i in range(g):
                        st = stg * g + i
                        xn = xnp.tile([128, h], F32, tag="xn", name="xn")
                        nc.sync.dma_start(xn, x_dram[st * 128 : (st + 1) * 128, :])
                        xns.append(xn)
                    for ht in range(ht_n):
                        ps = xps.tile([128, g * 128], F32, tag="xt")
                        for i in range(g):
                            nc.tensor.transpose(
                                ps[:, i * 128 : (i + 1) * 128],
                                xns[i][:, ht * 128 : (ht + 1) * 128],
                                id_sb,
                            )
                        nc.scalar.copy(
                            xT[ht][:, stg * g * 128 : (stg + 1) * g * 128], ps
                        )
                if which in ("q", "k"):
                    dst = qTh if which == "q" else kTh
                    bias = bq_sb if which == "q" else bk_sb
                    for ot in range(ot_n):
                        for sci in range(nsc):
                            ps = pps.tile([128, sc], F32, tag="proj")
                            for ht in range(ht_n):
                                nc.tensor.matmul(
                                    ps,
                                    wT[ht][:, ot * 128 : (ot + 1) * 128],
                                    xT[ht][:, sci * sc : (sci + 1) * sc],
                                    start=(ht == 0),
                                    stop=(ht == ht_n - 1),
                                )
                            for half in (0, 1):
                                nc.vector.tensor_scalar_add(
                                    dst[2 * ot + half][0:64, sci * sc : (sci + 1) * sc],
                                    ps[64 * half : 64 * half + 64, :],
                                    bias[ot][64 * half : 64 * half + 64, 0:1],
                                )
                else:
                    for st in range(st_n):
                        ps = pps.tile([128, oh], F32, tag="vproj")
                        for ht in range(ht_n):
                            nc.tensor.matmul(
                                ps,
                                xT[ht][:, st * 128 : (st + 1) * 128],
                                wT[ht],
                                start=(ht == 0),
                                stop=False,
                            )
                        nc.tensor.matmul(
                            ps,
                            ones_sb,
                            bv_r,
                            start=False,
                            stop=True,
                        )
                        nc.scalar.copy(v_sb[st], ps)

    # ---- attention
    with (
        tc.tile_pool(name="scp", bufs=3, space="PSUM") as scp,  # scores scratch
        tc.tile_pool(name="pcx", bufs=2, space="PSUM") as pcx,  # context / out proj
        tc.tile_pool(name="asb", bufs=3) as apool,
        tc.tile_pool(name="zp", bufs=1) as zpool,
    ):
        rzt = [zpool.tile([128, st_n], F32, tag=f"rzt{hh}", name=f"rzt{hh}") for hh in range(nhl)]
        zth = [zpool.tile([128, 2 * st_n], F32, tag=f"zt{hh}", name=f"zt{hh}") for hh in range(nhl)]
        zt = [zpool.tile([128, st_n], F32, tag=f"ztt{hh}", name=f"ztt{hh}") for hh in range(nhl)]
        hq = s // 2 if s >= 1024 else s  # natural-scores row piece per exp
        nhq = s // hq
        for qci in range(nqc):
            for hh in range(nhl):
                ot, off = (hh * hd) // 128, (hh * hd) % 128
                qh = qTh[hh]
                kh = kTh[hh]
                # C unit: natural scores -> softmax -> attention out (per q-tile)
                def c_unit(qtl, qh=qh, kh=kh, hh=hh, qci=qci):
                    qt = qci * qt_per_c + qtl
                    e = apool.tile([128, s], F32, tag="E", name="e")
                    for half in range(nhq):
                        sn = scp.tile([128, hq], F32, tag="sc", name="sn")
                        for kc in range(hq // sc):
                            kcg = half * (hq // sc) + kc
                            nc.tensor.matmul(
                                sn[:, kc * sc : (kc + 1) * sc],
                                qh[:, qt * 128 : (qt + 1) * 128],
                                kh[:, kcg * sc : (kcg + 1) * sc],
                                start=True,
                                stop=True,
                            )
                        nc.scalar.activation(
                            e[:, half * hq : (half + 1) * hq],
                            sn,
                            EXP,
                            scale=scale,
                            accum_out=zth[hh][:, 2 * qt + half : 2 * qt + half + 1],
                        )
                    if nhq == 2:
                        nc.vector.tensor_add(
                            zt[hh][:, qt : qt + 1],
                            zth[hh][:, 2 * qt : 2 * qt + 1],
                            zth[hh][:, 2 * qt + 1 : 2 * qt + 2],
                        )
                        rz_src = zt[hh][:, qt : qt + 1]
                    else:
                        rz_src = zth[hh][:, 2 * qt : 2 * qt + 1]
                    nc.vector.reciprocal(rzt[hh][:, qt : qt + 1], rz_src)
                    p = apool.tile([128, s], F32, tag="P", name="p")
                    nc.gpsimd.tensor_scalar_mul(p, e, rzt[hh][:, qt : qt + 1])
                    nc.sync.dma_start(att[hh, qt * 128 : (qt + 1) * 128, :], p)

                # A unit: transposed scores -> exp -> attention @ V (per k-pair)
                def a_unit(kp, cx, qh=qh, kh=kh, hh=hh, qci=qci):
                    stp = scp.tile([128, 2 * qc], F32, tag="sc", name="stp")
                    for j in (0, 1):
                        kt = kp * 2 + j
                        nc.tensor.matmul(
                            stp[:, j * qc : (j + 1) * qc],
                            kh[:, kt * 128 : (kt + 1) * 128],
                            qh[:, qci * qc : (qci + 1) * qc],
                            start=True,
                            stop=True,
                        )
                    et = apool.tile([128, 2 * qc], F32R, tag="ET", name="et")
                    nc.scalar.activation(et, stp, EXP, scale=scale)
                    for j in (0, 1):
                        kt = kp * 2 + j
                        nc.tensor.matmul(
                            cx[0:hd, 0:qc],
                            v_sb[kt][:, hh * hd : (hh + 1) * hd],
                            et[:, j * qc : (j + 1) * qc],
                            start=(kt == 0),
                            stop=(kt == kt_n - 1),
                        )

                n_c = qt_per_c if "C" not in skip else 0
                n_a = kt_n // 2 if "A" not in skip else 0
                cx = pcx.tile([128, max(qc, h)], F32, tag="cx", name="cx") if n_a else None
                a_done = 0
                for qtl in range(n_c):
                    c_unit(qtl)
                    a_target = (qtl + 1) * n_a // max(n_c, 1)
                    while a_done < a_target:
                        a_unit(a_done, cx)
                        a_done += 1
                while a_done < n_a:
                    a_unit(a_done, cx)
                    a_done += 1
                if "A" in skip:
                    continue
                if "C" in skip:
                    # no softmax stats available; store unnormalized context
                    nc.vector.tensor_copy(
                        ctx[ot][off : off + hd, qci * qc : (qci + 1) * qc],
                        cx[0:hd, 0:qc],
                    )
                    continue
                # build 1/Z as a row vector and normalize the context
                bc = pcx.tile([128, max(qc, h)], F32, tag="cx", name="bc")
                for qtl in range(qt_per_c):
                    qt = qci * qt_per_c + qtl
                    nc.tensor.transpose(
                        bc[0:1, qtl * 128 : (qtl + 1) * 128],
                        rzt[hh][:, qt : qt + 1],
                        id_sb,
                    )
                rzn = apool.tile([1, qc], F32R, tag="rzn")
                nc.vector.tensor_copy(rzn, bc[0:1, 0:qc])
                # broadcast 1/Z across hd partitions via a K=1 matmul
                nc.tensor.matmul(
                    bc[0:hd, 0:qc],
                    ones_sb[:, 0:hd],
                    rzn,
                    start=True,
                    stop=True,
                )
                rzf = apool.tile([hd, qc], F32, tag="rzf")
                nc.vector.tensor_copy(rzf, bc[0:hd, 0:qc])
                nc.vector.tensor_mul(
                    ctx[ot][off : off + hd, qci * qc : (qci + 1) * qc],
                    cx[0:hd, 0:qc],
                    rzf,
                )
            # output projection for this q-chunk
            for qtl in range(qt_per_c if ("O" not in skip and "A" not in skip) else 0):
                qt = qci * qt_per_c + qtl
                op = pcx.tile([128, max(qc, h)], F32, tag="cx")
                for ot2 in range(ot_n):
                    nc.tensor.matmul(
                        op[:, 0:h],
                        ctx[ot2][:, qt * 128 : (qt + 1) * 128],
                        woT[ot2],
                        start=(ot2 == 0),
                        stop=(ot2 == ot_n - 1),
                    )
                ob = apool.tile([128, h], F32, tag="OB")
                nc.vector.tensor_copy(ob, op[:, 0:h])
                nc.sync.dma_start(outp[qt * 128 : (qt + 1) * 128, :], ob)

    ppool.release()
    cpool.release()


_compiled = {}


def get_compiled(s=S, h=H, oh=OH):
    key = (s, h, oh)
    if key not in _compiled:
        nc = bacc.Bacc("TRN2", debug=False, enable_asserts=False, num_devices=NCORES)
        with tile.TileContext(nc) as tc:
            build_attn(nc, tc, s, h, oh)
        nc.compile()
        _compiled[key] = nc
    return _compiled[key]


def make_in_maps(query, key_in, value, Wq, bq, Wk, bk, Wv, bv, Wo, bo):
    """Shard full inputs into per-core input maps."""
    ident = np.eye(128, dtype=np.float32)
    in_maps = []
    for c in range(NCORES):
        b = c // 2
        g = c % 2
        o0, o1 = g * OH, (g + 1) * OH
        in_maps.append(
            {
                "xq": np.ascontiguousarray(query[b]),
                "xk": np.ascontiguousarray(key_in[b]),
                "xv": np.ascontiguousarray(value[b]),
                "wq": np.ascontiguousarray(Wq[o0:o1]),
                "wk": np.ascontiguousarray(Wk[o0:o1]),
                "wv": np.ascontiguousarray(Wv[o0:o1]),
                "wo": np.ascontiguousarray(Wo[:, o0:o1]),
                "bq": np.ascontiguousarray(bq[o0:o1]),
                "bk": np.ascontiguousarray(bk[o0:o1]),
                "bv": np.ascontiguousarray(bv[o0:o1]),
                "ident": ident,
            }
        )
    return in_maps


def assemble(results, bo):
    """Gather per-core outputs into (output, attention)."""
    attention = np.empty((B, NH, S, S), dtype=np.float32)
    output = np.empty((B, S, H), dtype=np.float32)
    for c in range(NCORES):
        b = c // 2
        g = c % 2
        attention[b, g * NHL : (g + 1) * NHL] = results[c]["att"]
    for b in range(B):
        output[b] = results[2 * b]["outp"] + results[2 * b + 1]["outp"] + bo
    return output, attention


def kernel(query, key_in, value, Wq, bq, Wk, bk, Wv, bv, Wo, bo):
    query = np.asarray(query, dtype=np.float32)
    key_in = np.asarray(key_in, dtype=np.float32)
    value = np.asarray(value, dtype=np.float32)
    Wq, bq = np.asarray(Wq, np.float32), np.asarray(bq, np.float32)
    Wk, bk = np.asarray(Wk, np.float32), np.asarray(bk, np.float32)
    Wv, bv = np.asarray(Wv, np.float32), np.asarray(bv, np.float32)
    Wo, bo = np.asarray(Wo, np.float32), np.asarray(bo, np.float32)

    nc = get_compiled()
    in_maps = make_in_maps(query, key_in, value, Wq, bq, Wk, bk, Wv, bv, Wo, bo)
    res = bass_utils.run_bass_kernel_spmd(nc, in_maps, list(range(NCORES)))
    return assemble(res.results, bo)


# revision 20
# speedup vs baseline: 5.1711x; 5.1711x over previous
"""Multi-head attention TRN2 Bass kernel (8-core SPMD).

Problem: nn_MultiHeadAttention — B=4, S=2048, H=512, NH=8, HD=64.
Returns (output [B,S,H], attention [B,NH,S,S]), matching the reference
nn.MultiheadAttention in eval mode.

Sharding: (batch, head-group) over 8 cores — core c handles batch c//2 and
heads (c%2)*4 .. (c%2)*4+4.  Each core computes its QKV projections for its
head slice, scores + softmax (exact fp32, written to DRAM), attention@V, and
a partial output projection.  Host sums the two partial output projections
per batch and adds bo.

Kernel internals (per core):
  - All matmuls run as float32r (full-rate fp32 path on the PE).
  - x and the weight slices are transposed on-chip via TensorE.
  - scores are computed twice: natural [q,k] orientation for the softmax /
    attention output (rowsum via ACT accum_out), and transposed [k,q]
    orientation (exp'd again) to feed attention@V with k on partitions.
  - context is produced transposed [o,q]; normalization 1/Z is applied via a
    partition-broadcast multiply, with 1/Z row vectors built by tiny TensorE
    transposes of the per-partition 1/Z columns.
"""

import os
import sys

import numpy as np

for _p in ("/opt/trn_rl_repo", "/root/.axon_site/_ro/trn_rl_repo"):
    if os.path.isdir(_p) and _p not in sys.path:
        sys.path.append(_p)

import concourse.bacc as bacc
import concourse.bass as bass
import concourse.tile as tile
from concourse import bass_utils, mybir

B, S, H, NH, HD = 4, 2048, 512, 8, 64
NCORES = 8
NHL = NH // 2  # heads per core
OH = NHL * HD  # per-core projection width (256)

F32 = mybir.dt.float32
F32R = mybir.dt.float32r
EXP = mybir.ActivationFunctionType.Exp


def declare_io(nc, s, h, oh):
    nhl = oh // HD
    return dict(
        xq=nc.dram_tensor("xq", [s, h], F32, kind="ExternalInput").ap(),
        xk=nc.dram_tensor("xk", [s, h], F32, kind="ExternalInput").ap(),
        xv=nc.dram_tensor("xv", [s, h], F32, kind="ExternalInput").ap(),
        wq=nc.dram_tensor("wq", [oh, h], F32, kind="ExternalInput").ap(),
        wk=nc.dram_tensor("wk", [oh, h], F32, kind="ExternalInput").ap(),
        wv=nc.dram_tensor("wv", [oh, h], F32, kind="ExternalInput").ap(),
        wo=nc.dram_tensor("wo", [h, oh], F32, kind="ExternalInput").ap(),
        bq=nc.dram_tensor("bq", [oh], F32, kind="ExternalInput").ap(),
        bk=nc.dram_tensor("bk", [oh], F32, kind="ExternalInput").ap(),
        bv=nc.dram_tensor("bv", [oh], F32, kind="ExternalInput").ap(),
        ident=nc.dram_tensor("ident", [128, 128], F32, kind="ExternalInput").ap(),
        att=nc.dram_tensor("att", [nhl, s, s], F32, kind="ExternalOutput").ap(),
        outp=nc.dram_tensor("outp", [s, h], F32, kind="ExternalOutput").ap(),
    )


def build_attn(nc, tc, s, h, oh, io=None, skip=()):
    """Emit the per-core attention program. s=seq len, h=model dim,
    oh=per-core head-slice width (NHL*64)."""
    hd = HD
    nhl = oh // hd
    st_n = s // 128  # s-tiles
    ht_n = h // 128  # h-tiles
    ot_n = oh // 128  # o-tiles
    qc = min(512, s)  # q-chunk for the transposed-scores path
    nqc = s // qc
    qt_per_c = qc // 128
    kt_n = s // 128
    sc = min(512, s)  # moving-dim chunk for projections / scores
    nsc = s // sc

    if io is None:
        io = declare_io(nc, s, h, oh)
    xq, xk, xv = io["xq"], io["xk"], io["xv"]
    wq, wk, wv, wo = io["wq"], io["wk"], io["wv"], io["wo"]
    bq, bk, bv = io["bq"], io["bk"], io["bv"]
    ident, att, outp = io["ident"], io["att"], io["outp"]

    scale = hd**-0.5

    cpool = tc.alloc_tile_pool(name="const", bufs=1)
    ppool = tc.alloc_tile_pool(name="persist", bufs=1)

    id_sb = cpool.tile([128, 128], F32, tag="ident")
    nc.sync.dma_start(id_sb, ident)
    ones_f = cpool.tile([1, 128], F32, tag="onesf")
    nc.vector.memset(ones_f, 1.0)
    ones_sb = cpool.tile([1, 128], F32R, tag="ones")
    nc.vector.tensor_copy(ones_sb, ones_f)
    bv_sb = cpool.tile([1, oh], F32, tag="bv")
    nc.sync.dma_start(bv_sb, bv[None, :])
    bv_r = cpool.tile([1, oh], F32R, tag="bvr")
    nc.vector.tensor_copy(bv_r, bv_sb)
    bq_sb = []
    bk_sb = []
    for ot in range(ot_n):
        tq = cpool.tile([128, 1], F32, tag=f"bq{ot}", name=f"bq{ot}")
        nc.sync.dma_start(tq, bq[ot * 128 : (ot + 1) * 128][:, None])
        bq_sb.append(tq)
        tk = cpool.tile([128, 1], F32, tag=f"bk{ot}", name=f"bk{ot}")
        nc.sync.dma_start(tk, bk[ot * 128 : (ot + 1) * 128][:, None])
        bk_sb.append(tk)

    # Persistent SBUF: projected tensors + context.
    # qTh[h]/kTh[h] are [128, s] with the head's 64 dims in rows 0:64 and
    # zeros in rows 64:128, so score matmuls contract over K=128 (2x faster
    # on the PE than K=64).
    qTh = [ppool.tile([128, s], F32R, tag=f"qTh{i}", name=f"qTh{i}") for i in range(nhl)]
    kTh = [ppool.tile([128, s], F32R, tag=f"kTh{i}", name=f"kTh{i}") for i in range(nhl)]
    zpad = cpool.tile([64, s], F32, tag="zpad")
    nc.vector.memset(zpad, 0.0)
    for i in range(nhl):
        nc.vector.tensor_copy(qTh[i][64:128, :], zpad)
        nc.vector.tensor_copy(kTh[i][64:128, :], zpad)
    v_sb = [ppool.tile([128, oh], F32R, tag=f"v{t}", name=f"v{t}") for t in range(st_n)]
    ctx = [ppool.tile([128, s], F32R, tag=f"ctx{t}", name=f"ctx{t}") for t in range(ot_n)]
    woT = [ppool.tile([128, h], F32R, tag=f"woT{t}", name=f"woT{t}") for t in range(ot_n)]

    # ---- weight transposes: wq/wk/wv [oh,h] -> [h,oh]; wo [h,oh] -> [oh,h]
    with (
        tc.tile_pool(name="wsb", bufs=2) as wpool,
        tc.tile_pool(name="wps", bufs=2, space="PSUM") as wpsum,
    ):
        wqT = [wpool.tile([128, oh], F32R, tag=f"wqT{t}", name=f"wqT{t}") for t in range(ht_n)]
        wkT = [wpool.tile([128, oh], F32R, tag=f"wkT{t}", name=f"wkT{t}") for t in range(ht_n)]
        wvT = [wpool.tile([128, oh], F32R, tag=f"wvT{t}", name=f"wvT{t}") for t in range(ht_n)]
        for w_dram, wT in ((wq, wqT), (wk, wkT), (wv, wvT)):
            for ot in range(ot_n):
                wn = wpool.tile([128, h], F32, tag="wn")
                nc.sync.dma_start(wn, w_dram[ot * 128 : (ot + 1) * 128, :])
                for ht in range(ht_n):
                    ps = wpsum.tile([128, 128], F32, tag="wt")
                    nc.tensor.transpose(ps, wn[:, ht * 128 : (ht + 1) * 128], id_sb)
                    nc.scalar.copy(wT[ht][:, ot * 128 : (ot + 1) * 128], ps)
        for ht in range(ht_n):
            wn = wpool.tile([128, h], F32, tag="wn")
            nc.sync.dma_start(wn[:, :oh], wo[ht * 128 : (ht + 1) * 128, :])
            for ot in range(ot_n):
                ps = wpsum.tile([128, 128], F32, tag="wt")
                nc.tensor.transpose(ps, wn[:, ot * 128 : (ot + 1) * 128], id_sb)
                nc.scalar.copy(woT[ot][:, ht * 128 : (ht + 1) * 128], ps)

        # ---- x transpose + projections (one input tensor at a time)
        for which, x_dram, wT in (("q", xq, wqT), ("k", xk, wkT), ("v", xv, wvT)):
            with (
                tc.tile_pool(name=f"xT_{which}", bufs=1) as xtp,
                tc.tile_pool(name=f"xn_{which}", bufs=8) as xnp,
                tc.tile_pool(name=f"xps_{which}", bufs=3, space="PSUM") as xps,
                tc.tile_pool(name=f"pps_{which}", bufs=2, space="PSUM") as pps,
            ):
                xT = [xtp.tile([128, s], F32R, tag=f"xT{t}", name=f"xT{t}") for t in range(ht_n)]
                g = min(4, st_n)
                for stg in range(st_n // g):
                    xns = []
                    for i in range(g):
                        st = stg * g + i
                        xn = xnp.tile([128, h], F32, tag="xn", name="xn")
                        nc.sync.dma_start(xn, x_dram[st * 128 : (st + 1) * 128, :])
                        xns.append(xn)
                    for ht in range(ht_n):
                        ps = xps.tile([128, g * 128], F32, tag="xt")
                        for i in range(g):
                            nc.tensor.transpose(
                                ps[:, i * 128 : (i + 1) * 128],
                                xns[i][:, ht * 128 : (ht + 1) * 128],
                                id_sb,
                            )
                        nc.scalar.copy(
                            xT[ht][:, stg * g * 128 : (stg + 1) * g * 128], ps
                        )
                if which in ("q", "k"):
                    dst = qTh if which == "q" else kTh
                    bias = bq_sb if which == "q" else bk_sb
                    for ot in range(ot_n):
                        for sci in range(nsc):
                            ps = pps.tile([128, sc], F32, tag="proj")
                            for ht in range(ht_n):
                                nc.tensor.matmul(
                                    ps,
                                    wT[ht][:, ot * 128 : (ot + 1) * 128],
                                    xT[ht][:, sci * sc : (sci + 1) * sc],
                                    start=(ht == 0),
                                    stop=(ht == ht_n - 1),
                                )
                            for half in (0, 1):
                                nc.vector.tensor_scalar_add(
                                    dst[2 * ot + half][0:64, sci * sc : (sci + 1) * sc],
                                    ps[64 * half : 64 * half + 64, :],
                                    bias[ot][64 * half : 64 * half + 64, 0:1],
                                )
                else:
                    for st in range(st_n):
                        ps = pps.tile([128, oh], F32, tag="vproj")
                        for ht in range(ht_n):
                            nc.tensor.matmul(
                                ps,
                                xT[ht][:, st * 128 : (st + 1) * 128],
                                wT[ht],
                                start=(ht == 0),
                                stop=False,
                            )
                        nc.tensor.matmul(
                            ps,
                            ones_sb,
                            bv_r,
                            start=False,
                            stop=True,
                        )
                        nc.scalar.copy(v_sb[st], ps)

    # ---- attention
    with (
        tc.tile_pool(name="scp", bufs=3, space="PSUM") as scp,  # scores scratch
        tc.tile_pool(name="pcx", bufs=2, space="PSUM") as pcx,  # context / out proj
        tc.tile_pool(name="asb", bufs=3) as apool,
        tc.tile_pool(name="zp", bufs=1) as zpool,
    ):
        rzt = [zpool.tile([128, st_n], F32, tag=f"rzt{hh}", name=f"rzt{hh}") for hh in range(nhl)]
        zth = [zpool.tile([128, 2 * st_n], F32, tag=f"zt{hh}", name=f"zt{hh}") for hh in range(nhl)]
        zt = [zpool.tile([128, st_n], F32, tag=f"ztt{hh}", name=f"ztt{hh}") for hh in range(nhl)]
        hq = s // 2 if s >= 1024 else s  # natural-scores row piece per exp
        nhq = s // hq
        for qci in range(nqc):
            for hh in range(nhl):
                ot, off = (hh * hd) // 128, (hh * hd) % 128
                qh = qTh[hh]
                kh = kTh[hh]
                # C unit: natural scores -> softmax -> attention out (per q-tile)
                def c_unit(qtl, qh=qh, kh=kh, hh=hh, qci=qci):
                    qt = qci * qt_per_c + qtl
                    e = apool.tile([128, s], F32, tag="E", name="e")
                    for half in range(nhq):
                        sn = scp.tile([128, hq], F32, tag="sc", name="sn")
                        for kc in range(hq // sc):
                            kcg = half * (hq // sc) + kc
                            nc.tensor.matmul(
                                sn[:, kc * sc : (kc + 1) * sc],
                                qh[:, qt * 128 : (qt + 1) * 128],
                                kh[:, kcg * sc : (kcg + 1) * sc],
                                start=True,
                                stop=True,
                            )
                        nc.scalar.activation(
                            e[:, half * hq : (half + 1) * hq],
                            sn,
                            EXP,
                            scale=scale,
                            accum_out=zth[hh][:, 2 * qt + half : 2 * qt + half + 1],
                        )
                    if nhq == 2:
                        nc.vector.tensor_add(
                            zt[hh][:, qt : qt + 1],
                            zth[hh][:, 2 * qt : 2 * qt + 1],
                            zth[hh][:, 2 * qt + 1 : 2 * qt + 2],
                        )
                        rz_src = zt[hh][:, qt : qt + 1]
                    else:
                        rz_src = zth[hh][:, 2 * qt : 2 * qt + 1]
                    nc.vector.reciprocal(rzt[hh][:, qt : qt + 1], rz_src)
                    p = apool.tile([128, s], F32, tag="P", name="p")
                    nc.vector.tensor_scalar_mul(p, e, rzt[hh][:, qt : qt + 1])
                    nc.sync.dma_start(att[hh, qt * 128 : (qt + 1) * 128, :], p)

                # A unit: transposed scores -> exp -> attention @ V (per k-pair)
                def a_unit(kp, cx, qh=qh, kh=kh, hh=hh, qci=qci):
                    stp = scp.tile([128, 2 * qc], F32, tag="sc", name="stp")
                    for j in (0, 1):
                        kt = kp * 2 + j
                        nc.tensor.matmul(
                            stp[:, j * qc : (j + 1) * qc],
                            kh[:, kt * 128 : (kt + 1) * 128],
                            qh[:, qci * qc : (qci + 1) * qc],
                            start=True,
                            stop=True,
                        )
                    et = apool.tile([128, 2 * qc], F32R, tag="ET", name="et")
                    nc.scalar.activation(et, stp, EXP, scale=scale)
                    for j in (0, 1):
                        kt = kp * 2 + j
                        nc.tensor.matmul(
                            cx[0:hd, 0:qc],
                            v_sb[kt][:, hh * hd : (hh + 1) * hd],
                            et[:, j * qc : (j + 1) * qc],
                            start=(kt == 0),
                            stop=(kt == kt_n - 1),
                        )

                n_c = qt_per_c if "C" not in skip else 0
                n_a = kt_n // 2 if "A" not in skip else 0
                cx = pcx.tile([128, max(qc, h)], F32, tag="cx", name="cx") if n_a else None
                a_done = 0
                for qtl in range(n_c):
                    c_unit(qtl)
                    a_target = (qtl + 1) * n_a // max(n_c, 1)
                    while a_done < a_target:
                        a_unit(a_done, cx)
                        a_done += 1
                while a_done < n_a:
                    a_unit(a_done, cx)
                    a_done += 1
                if "A" in skip:
                    continue
                if "C" in skip:
                    # no softmax stats available; store unnormalized context
                    nc.vector.tensor_copy(
                        ctx[ot][off : off + hd, qci * qc : (qci + 1) * qc],
                        cx[0:hd, 0:qc],
                    )
                    continue
                # build 1/Z as a row vector and normalize the context
                bc = pcx.tile([128, max(qc, h)], F32, tag="cx", name="bc")
                for qtl in range(qt_per_c):
                    qt = qci * qt_per_c + qtl
                    nc.tensor.transpose(
                        bc[0:1, qtl * 128 : (qtl + 1) * 128],
                        rzt[hh][:, qt : qt + 1],
                        id_sb,
                    )
                rzn = apool.tile([1, qc], F32R, tag="rzn")
                nc.vector.tensor_copy(rzn, bc[0:1, 0:qc])
                # broadcast 1/Z across hd partitions via a K=1 matmul
                nc.tensor.matmul(
                    bc[0:hd, 0:qc],
                    ones_sb[:, 0:hd],
                    rzn,
                    start=True,
                    stop=True,
                )
                rzf = apool.tile([hd, qc], F32, tag="rzf")
                nc.vector.tensor_copy(rzf, bc[0:hd, 0:qc])
                nc.vector.tensor_mul(
                    ctx[ot][off : off + hd, qci * qc : (qci + 1) * qc],
                    cx[0:hd, 0:qc],
                    rzf,
                )
            # output projection for this q-chunk
            for qtl in range(qt_per_c if ("O" not in skip and "A" not in skip) else 0):
                qt = qci * qt_per_c + qtl
                op = pcx.tile([128, max(qc, h)], F32, tag="cx")
                for ot2 in range(ot_n):
                    nc.tensor.matmul(
                        op[:, 0:h],
                        ctx[ot2][:, qt * 128 : (qt + 1) * 128],
                        woT[ot2],
                        start=(ot2 == 0),
                        stop=(ot2 == ot_n - 1),
                    )
                ob = apool.tile([128, h], F32, tag="OB")
                nc.vector.tensor_copy(ob, op[:, 0:h])
                nc.sync.dma_start(outp[qt * 128 : (qt + 1) * 128, :], ob)

    ppool.release()
    cpool.release()


_compiled = {}


def get_compiled(s=S, h=H, oh=OH):
    key = (s, h, oh)
    if key not in _compiled:
        nc = bacc.Bacc("TRN2", debug=False, enable_asserts=False, num_devices=NCORES)
        with tile.TileContext(nc) as tc:
            build_attn(nc, tc, s, h, oh)
        nc.compile()
        _compiled[key] = nc
    return _compiled[key]


def make_in_maps(query, key_in, value, Wq, bq, Wk, bk, Wv, bv, Wo, bo):
    """Shard full inputs into per-core input maps."""
    ident = np.eye(128, dtype=np.float32)
    in_maps = []
    for c in range(NCORES):
        b = c // 2
        g = c % 2
        o0, o1 = g * OH, (g + 1) * OH
        in_maps.append(
            {
                "xq": np.ascontiguousarray(query[b]),
                "xk": np.ascontiguousarray(key_in[b]),
                "xv": np.ascontiguousarray(value[b]),
                "wq": np.ascontiguousarray(Wq[o0:o1]),
                "wk": np.ascontiguousarray(Wk[o0:o1]),
                "wv": np.ascontiguousarray(Wv[o0:o1]),
                "wo": np.ascontiguousarray(Wo[:, o0:o1]),
                "bq": np.ascontiguousarray(bq[o0:o1]),
                "bk": np.ascontiguousarray(bk[o0:o1]),
                "bv": np.ascontiguousarray(bv[o0:o1]),
                "ident": ident,
            }
        )
    return in_maps


def assemble(results, bo):
    """Gather per-core outputs into (output, attention)."""
    attention = np.empty((B, NH, S, S), dtype=np.float32)
    output = np.empty((B, S, H), dtype=np.float32)
    for c in range(NCORES):
        b = c // 2
        g = c % 2
        attention[b, g * NHL : (g + 1) * NHL] = results[c]["att"]
    for b in range(B):
        output[b] = results[2 * b]["outp"] + results[2 * b + 1]["outp"] + bo
    return output, attention


def kernel(query, key_in, value, Wq, bq, Wk, bk, Wv, bv, Wo, bo):
    query = np.asarray(query, dtype=np.float32)
    key_in = np.asarray(key_in, dtype=np.float32)
    value = np.asarray(value, dtype=np.float32)
    Wq, bq = np.asarray(Wq, np.float32), np.asarray(bq, np.float32)
    Wk, bk = np.asarray(Wk, np.float32), np.asarray(bk, np.float32)
    Wv, bv = np.asarray(Wv, np.float32), np.asarray(bv, np.float32)
    Wo, bo = np.asarray(Wo, np.float32), np.asarray(bo, np.float32)

    nc = get_compiled()
    in_maps = make_in_maps(query, key_in, value, Wq, bq, Wk, bk, Wv, bv, Wo, bo)
    res = bass_utils.run_bass_kernel_spmd(nc, in_maps, list(range(NCORES)))
    return assemble(res.results, bo)
